# revision 4
# baseline (speedup 1.0000x reference)
"""Trainium2 Bass kernel for nn_ExtSummarizer (B=512, S=100, H=768).

Math (per batch b, mask==1, true_dim==S):
  cont[i] = W_cont . s_i
  sim[i,j] = s_i W_sim s_j^T
  d = mean_i s_i ;  rel[i] = s_i . (W_rel d)
  off[i] = rel[i] + cont[i] + b = s_i . u + b,  u = W_rel d + W_cont^T
  q = sigmoid(sim + off[:,None])
  sv[j] = sum_i q[i,j];  solve (I - lam*q*diag(1/sv)) x = y,  y = 1/S
  score = (1-lam) x

Device algorithm:
  - row-scaled operator: At = lam * diag(1/sv) q  (per-partition scale)
    x = diag(sv) z,  z = sum_{k<2^J} At^k y0,  y0 = (1/S)*(1/sv)
  - z via doubling: z_{j+1} = z_j + A_j z_j, A_{j+1} = A_j^2 (A and A^T both
    kept in SBUF so squaring needs no transposes:
      [A^2 | A z] = matmul(lhsT=A^T, rhs=[A | z]);  (A^2)^T = matmul(lhsT=A, rhs=A^T))

Sharding: pure data parallel, 64 batches per core, 8 cores.
All matmuls fp16 operands with fp32 PSUM accumulate.
"""

import numpy as np

B, S, H = 512, 100, 768
NCORES = 8
BC = B // NCORES          # 64 batches per core
ROWS = BC * S             # 6400 rows per core
LAMB = 0.8
NGRP = 2                  # row-groups per core
GB = BC // NGRP           # 32 batches per group
GROWS = GB * S            # 3200 rows per group
HC = H // 128             # 6 k-chunks
NT = 400                  # mm1 moving-dim tile (8 * 400 = 3200)
NNT = GROWS // NT
J = 6                     # Neumann doubling steps: covers k < 2^J = 64

_CACHE = {}


def _get_nc():
    if "nc" in _CACHE:
        return _CACHE["nc"]

    import concourse.bass as bass
    import concourse.mybir as mybir
    import concourse.tile as tile
    from concourse import bacc
    from concourse.bass import ts

    fp16 = mybir.dt.float16
    fp32 = mybir.dt.float32
    AF = mybir.ActivationFunctionType
    OP = mybir.AluOpType
    X = mybir.AxisListType.X

    nc = bacc.Bacc(trn_type="TRN2", target_bir_lowering=False, debug=False)

    sent16 = nc.dram_tensor("sent16", [ROWS, H], fp16, kind="ExternalInput")
    wsim16 = nc.dram_tensor("wsim16", [H, H], fp16, kind="ExternalInput")
    wrelT16 = nc.dram_tensor("wrelT16", [H, H], fp16, kind="ExternalInput")
    wcont32 = nc.dram_tensor("wcont32", [H], fp32, kind="ExternalInput")
    ones16 = nc.dram_tensor("ones16", [S, 1], fp16, kind="ExternalInput")
    eye16 = nc.dram_tensor("eye16", [S, S], fp16, kind="ExternalInput")
    bvec32 = nc.dram_tensor("bvec32", [S, 1], fp32, kind="ExternalInput")
    out32 = nc.dram_tensor("out32", [BC, S], fp32, kind="ExternalOutput")

    with tile.TileContext(nc) as tc:
        with (
            tc.tile_pool(name="const", bufs=1) as const,
            tc.tile_pool(name="sentT_p", bufs=2) as sentT_p,
            tc.tile_pool(name="yt_p", bufs=2) as yt_p,
            tc.tile_pool(name="grp_p", bufs=2) as grp_p,
            tc.tile_pool(name="small", bufs=3) as small,
            tc.tile_pool(name="ps_mm", bufs=2, space="PSUM") as ps_mm,
            tc.tile_pool(name="ps_sim", bufs=1, space="PSUM") as ps_sim,
            tc.tile_pool(name="ps_misc", bufs=2, space="PSUM") as ps_misc,
            tc.tile_pool(name="ps_sq", bufs=2, space="PSUM") as ps_sq,
            tc.tile_pool(name="ps_sqT", bufs=1, space="PSUM") as ps_sqT,
        ):
            wsim_sb = const.tile([128, HC, H], fp16)
            nc.sync.dma_start(
                wsim_sb[:], wsim16.ap().rearrange("(c p) n -> p c n", p=128)
            )
            wrelT_sb = const.tile([128, HC, H], fp16)
            nc.sync.dma_start(
                wrelT_sb[:], wrelT16.ap().rearrange("(c p) n -> p c n", p=128)
            )
            wcont_sb = const.tile([128, HC], fp32)
            nc.sync.dma_start(
                wcont_sb[:], wcont32.ap().rearrange("(c p) -> p c", p=128)
            )
            ones_sb = const.tile([S, 1], fp16)
            nc.sync.dma_start(ones_sb[:], ones16.ap())
            eye_sb = const.tile([S, S], fp16)
            nc.sync.dma_start(eye_sb[:], eye16.ap())
            bvec_sb = const.tile([S, 1], fp32)
            nc.sync.dma_start(bvec_sb[:], bvec32.ap())

            for g in range(NGRP):
                # --- transposed load: sentT[p, c, r] = sent[row r, h=c*128+p]
                sentT = sentT_p.tile([128, HC, GROWS], fp16, tag="sentT")
                for c in range(HC):
                    nc.sync.dma_start(
                        out=sentT[:, c, :],
                        in_=sent16.ap()[
                            g * GROWS : (g + 1) * GROWS, c * 128 : (c + 1) * 128
                        ],
                        transpose=True,
                    )

                # --- d = mean_i s_i  (per batch), then u = W_rel d + W_cont^T
                d32 = grp_p.tile([128, HC, GB], fp32, tag="d32")
                for c in range(HC):
                    nc.vector.reduce_sum(
                        out=d32[:, c, :],
                        in_=sentT[:, c, :].rearrange("p (b s) -> p b s", s=S),
                        axis=X,
                    )
                d16 = grp_p.tile([128, HC, GB], fp16, tag="d16")
                nc.vector.tensor_scalar(
                    out=d16[:], in0=d32[:], scalar1=1.0 / S, scalar2=None, op0=OP.mult
                )
                u16 = grp_p.tile([128, HC, GB], fp16, tag="u16")
                for m in range(HC):
                    psu = ps_mm.tile([128, NT], fp32, tag="mm")
                    for c in range(HC):
                        nc.tensor.matmul(
                            psu[:, :GB],
                            wrelT_sb[:, c, m * 128 : (m + 1) * 128],
                            d16[:, c, :],
                            start=(c == 0),
                            stop=(c == HC - 1),
                        )
                    nc.vector.tensor_scalar(
                        out=u16[:, m, :],
                        in0=psu[:, :GB],
                        scalar1=wcont_sb[:, m : m + 1],
                        scalar2=None,
                        op0=OP.add,
                    )

                # --- mm1: YT[p, m, r] = (sent @ W_sim)^T in the same chunked layout
                yt = yt_p.tile([128, HC, GROWS], fp16, tag="yt")
                for m in range(HC):
                    for n in range(NNT):
                        psy = ps_mm.tile([128, NT], fp32, tag="mm")
                        for c in range(HC):
                            nc.tensor.matmul(
                                psy[:],
                                wsim_sb[:, c, m * 128 : (m + 1) * 128],
                                sentT[:, c, ts(n, NT)],
                                start=(c == 0),
                                stop=(c == HC - 1),
                            )
                        if (m * NNT + n) % 2 == 0:
                            nc.scalar.copy(yt[:, m, ts(n, NT)], psy[:])
                        else:
                            nc.vector.tensor_copy(yt[:, m, ts(n, NT)], psy[:])

                # --- per-batch: scores, sigmoid, column sums, Neumann solve
                xg = grp_p.tile([S, GB], fp32, tag="xg")
                for bl in range(GB):
                    sl = slice(bl * S, (bl + 1) * S)
                    # sim[i, j]
                    ps_s = ps_sim.tile([S, S], fp32, tag="sim")
                    for c in range(HC):
                        nc.tensor.matmul(
                            ps_s[:],
                            yt[:, c, sl],
                            sentT[:, c, sl],
                            start=(c == 0),
                            stop=(c == HC - 1),
                        )
                    # off[i] (col 0), sv (col 1), At fp16 scratch (bytes 16..216)
                    psm = ps_misc.tile([128, 512], fp32, tag="misc")
                    off_ps = psm[:S, 0:1]
                    sv_ps = psm[:S, 1:2]
                    for c in range(HC):
                        nc.tensor.matmul(
                            off_ps,
                            sentT[:, c, sl],
                            u16[:, c, bl : bl + 1],
                            start=(c == 0),
                            stop=(c == HC - 1),
                        )
                    off_sb = small.tile([S, 1], fp32, tag="off")
                    nc.vector.tensor_scalar(
                        out=off_sb[:],
                        in0=off_ps,
                        scalar1=bvec_sb[:, 0:1],
                        scalar2=None,
                        op0=OP.add,
                    )
                    q_sb = small.tile([S, S], fp16, tag="q")
                    nc.scalar.activation(
                        q_sb[:], ps_s[:], AF.Sigmoid, bias=off_sb[:, 0:1], scale=1.0
                    )
                    # sv[j] = sum_i q[i, j]  -> [S, 1] on partitions
                    nc.tensor.matmul(sv_ps, q_sb[:], ones_sb[:], start=True, stop=True)
                    r_sb = small.tile([S, 1], fp32, tag="r")
                    nc.vector.reciprocal(r_sb[:], sv_ps)
                    sv_sb = small.tile([S, 1], fp32, tag="sv")
                    nc.scalar.mul(sv_sb[:], sv_ps, 1.0 - LAMB)  # 0.2*sv
                    # A0 = lam * r * q with z-col y0 = (1/S) * r appended
                    A_cur = small.tile([S, S + 1], fp16, tag="Achain")
                    nc.vector.tensor_scalar(
                        out=A_cur[:, 0:S],
                        in0=q_sb[:],
                        scalar1=r_sb[:, 0:1],
                        scalar2=LAMB,
                        op0=OP.mult,
                        op1=OP.mult,
                    )
                    nc.vector.tensor_scalar(
                        out=A_cur[:, S : S + 1],
                        in0=r_sb[:, 0:1],
                        scalar1=1.0 / S,
                        scalar2=None,
                        op0=OP.mult,
                    )
                    At_ps = psm[:S, 4:54].bitcast(fp16)  # [S, 100] fp16 view
                    nc.tensor.transpose(At_ps, A_cur[:, 0:S], eye_sb[:])
                    AT_sb = small.tile([S, S], fp16, tag="ATchain")
                    nc.scalar.copy(AT_sb[:], At_ps)
                    for j in range(J):
                        if j < J - 1:
                            sq = ps_sq.tile([S, S + 1], fp32, tag="sq")
                            nc.tensor.matmul(
                                sq[:], AT_sb[:], A_cur[:], start=True, stop=True
                            )  # [A^2 | A z]
                            sqT = ps_sqT.tile([S, S], fp32, tag="sqT")
                            nc.tensor.matmul(
                                sqT[:], A_cur[:, 0:S], AT_sb[:], start=True, stop=True
                            )  # (A^2)^T
                            A_next = small.tile([S, S + 1], fp16, tag="Achain")
                            nc.vector.tensor_copy(A_next[:, 0:S], sq[:, 0:S])
                            nc.vector.tensor_tensor(
                                out=A_next[:, S : S + 1],
                                in0=sq[:, S : S + 1],
                                in1=A_cur[:, S : S + 1],
                                op=OP.add,
                            )
                            AT_next = small.tile([S, S], fp16, tag="ATchain")
                            nc.scalar.copy(AT_next[:], sqT[:])
                            A_cur, AT_sb = A_next, AT_next
                        else:
                            sq = ps_sq.tile([S, S + 1], fp32, tag="sq")
                            nc.tensor.matmul(
                                sq[:, 0:1],
                                AT_sb[:],
                                A_cur[:, S : S + 1],
                                start=True,
                                stop=True,
                            )  # A z (final)
                            # x = 0.2 * sv * (z + Az)
                            zf = small.tile([S, 1], fp32, tag="zf")
                            nc.vector.tensor_tensor(
                                out=zf[:],
                                in0=sq[:, 0:1],
                                in1=A_cur[:, S : S + 1],
                                op=OP.add,
                            )
                            nc.vector.tensor_scalar(
                                out=xg[:, bl : bl + 1],
                                in0=zf[:],
                                scalar1=sv_sb[:, 0:1],
                                scalar2=None,
                                op0=OP.mult,
                            )

                nc.sync.dma_start(
                    out=out32.ap()[g * GB : (g + 1) * GB, :].rearrange("b s -> s b"),
                    in_=xg[:],
                )

    nc.compile()
    _CACHE["nc"] = nc
    return nc


def _prep(inputs):
    sent = np.ascontiguousarray(np.asarray(inputs["sent_vec"], dtype=np.float32))
    sent16 = sent.reshape(B * S, H).astype(np.float16).reshape(NCORES, ROWS, H)
    wsim16 = np.ascontiguousarray(
        np.asarray(inputs["W_sim"], dtype=np.float32)
    ).astype(np.float16)
    wrelT16 = np.ascontiguousarray(
        np.asarray(inputs["W_rel"], dtype=np.float32).T
    ).astype(np.float16)
    wcont = np.asarray(inputs["W_cont"], dtype=np.float32).reshape(H)
    bval = float(np.asarray(inputs["b_matrix"]).reshape(-1)[0])
    ones = np.ones((S, 1), np.float16)
    eye = np.eye(S, dtype=np.float16)
    bvec = np.full((S, 1), bval, np.float32)
    return [
        {
            "sent16": np.ascontiguousarray(sent16[i]),
            "wsim16": wsim16,
            "wrelT16": wrelT16,
            "wcont32": wcont,
            "ones16": ones,
            "eye16": eye,
            "bvec32": bvec,
        }
        for i in range(NCORES)
    ]


def _run(in_maps, trace=False, **kw):
    from concourse.bass_utils import run_bass_kernel_spmd

    nc = _get_nc()
    return run_bass_kernel_spmd(nc, in_maps, list(range(NCORES)), trace=trace, **kw)


def kernel(**inputs):
    in_maps = _prep(inputs)
    res = _run(in_maps)
    out = np.concatenate([r["out32"] for r in res.results], axis=0)
    return np.ascontiguousarray(out, dtype=np.float32)


if __name__ == "__main__":
    _get_nc()
    print("build ok")
